# revision 36
# baseline (speedup 1.0000x reference)
"""GCN message-passing kernel for 8 Trainium2 NeuronCores (Bass/Tile).

Strategy (self-contained; shapes hardcoded/derived from inputs):
- Nodes partitioned across 8 cores by graph (graph_ids is sorted -> contiguous
  node ranges, 16 graphs/core). Edges partitioned by dst owner.
- Node positions within a core are PERMUTED (host-side) to jointly load-balance
  the two gather streams across 128-node subwindows (greedy L2 on per-stream
  loads); every downstream array is permuted consistently.
- h is replicated on every core as an fp8-e4m3 [8*NPU, D] DRAM table
  (addr_space=Shared), refreshed by an AllGather each layer. Messages = bulk
  dma_gather of h[src] rows, split into L/H streams so int16 indices reach the
  whole table. Gathers round-robin over 4 SWDGE queues with a 32KB descriptor
  scratch so desc-gen pipelines instead of blocking on ring space.
- Per 128-edge tile, a one-hot Sel matrix (edge -> node-rank-in-subwindow,
  built by one broadcast is_equal, fp8) turns the segment-sum into an
  accumulating matmul into a node-indexed PSUM window; tiles are packed
  densely (node runs may split across tiles - PSUM accumulates). Self-edges
  fold the "+h" residual in. The bond-embedding term is a window-wide
  countT @ T512 matmul that also opens the PSUM accumulation (it does not
  depend on the gathers, so it hides gather latency).
- LAYER 1 IS FULLY FUSED: messages depend only on categorical codes, so
  neigh+self collapse into exact integer count-matrix matmuls
  (cntA[atom-code, dst] @ atom_emb + countT @ T512) - no h0 phase, no initial
  AllGather, no layer-1 gathers at all.
- deg-normalization multiplies by a host-shipped broadcast row; update matmul
  in bf16; pre-BN h stays in SBUF (hlinS). BatchNorm stats = masked column-sum
  matmuls, all-reduced (2x256 f32), applied as scale/shift broadcast rows with
  the pad-row mask folded into a scalar-engine Relu (mask rides the per-
  partition activation scale).
"""
import sys

sys.path.insert(0, "/opt/trn_rl_repo")

import heapq

import numpy as np
import ml_dtypes

import os

import concourse.bass as bass
import concourse.bacc as bacc
import concourse.mybir as mybir
import concourse.tile as tile
from concourse.bass_utils import run_bass_kernel_spmd
from concourse import library_config

P = 128
WSZ = 512          # psum node window
CH = 8             # gather tiles per bulk dma_gather
EPS = 1e-5
NCORES = 8
BF16 = mybir.dt.bfloat16
F32 = mybir.dt.float32
I16 = mybir.dt.int16
FP8 = mybir.dt.float8e4


# ----------------------------------------------------------------------------
# Host preprocessing
# ----------------------------------------------------------------------------

def _wrap_idx(flat):
    """int16 idx stream -> [128, n/16] wrapped layout (j -> (j%16, j//16)),
    replicated down the 128 partitions (8 Q7 cores x 16)."""
    n = flat.shape[0]
    assert n % 16 == 0
    w = flat.reshape(n // 16, 16).T.astype(np.int16)  # [16, n/16]
    return np.tile(w, (8, 1))


def preprocess(inputs, n_graphs=128):
    nfeat = np.asarray(inputs["nfeat"], np.int64)
    efeat = np.asarray(inputs["efeat"], np.int64)
    src = np.asarray(inputs["src"], np.int64)
    dst = np.asarray(inputs["dst"], np.int64)
    graph_ids = np.asarray(inputs["graph_ids"], np.int64)
    atom_emb = np.asarray(inputs["atom_emb"], np.float32)
    edge_emb = np.asarray(inputs["edge_emb"], np.float32)
    W = np.asarray(inputs["W"], np.float32)
    gamma = np.asarray(inputs["gamma"], np.float32)
    beta = np.asarray(inputs["beta"], np.float32)
    Wp = np.asarray(inputs["Wp"], np.float32)
    bp = np.asarray(inputs["bp"], np.float32)

    N = graph_ids.shape[0]
    E = src.shape[0]
    G = n_graphs
    GPC = G // NCORES
    AC, AV, D = atom_emb.shape
    L, BC, BV, _ = edge_emb.shape
    NCOMB = BV ** BC
    OUT = Wp.shape[1]
    HALF = NCORES // 2

    # graph -> core node ranges
    gcnt = np.bincount(graph_ids, minlength=G)
    gofs = np.concatenate([[0], np.cumsum(gcnt)])
    S = gofs[::GPC].astype(np.int64)            # len NCORES+1
    assert S[-1] == N
    Nc = np.diff(S)

    nwin = int(np.ceil((Nc.max() + 1) / WSZ))
    NPU = nwin * WSZ                            # padded nodes per core
    NSW = NPU // P                              # subwindows (= subtiles)
    B_SPLIT = HALF * NPU                        # hi gather-table offset
    assert B_SPLIT < 32768 and (NCORES - HALF) * NPU < 32768

    degs = np.bincount(dst, minlength=N).astype(np.float64) + 1.0
    deginv_all = (1.0 / degs).astype(np.float32)

    node_core = (np.searchsorted(S[1:], np.arange(N), side="right")).astype(np.int64)
    src_core = node_core[src]
    e_isL = src_core < HALF
    # stream degree per dst node (self edge counts in own-half stream)
    dLn = np.bincount(dst[e_isL], minlength=N) + (node_core < HALF)
    dHn = np.bincount(dst[~e_isL], minlength=N) + (node_core >= HALF)

    # --- per-core node permutation: balance (dL+dH) across NSW bins of 128 ---
    pos_of_node = np.full(N, -1, np.int64)      # global padded position
    node_at_pos = [np.full(NPU, -1, np.int64) for _ in range(NCORES)]
    for c in range(NCORES):
        nodes = np.arange(S[c], S[c + 1])
        dl = dLn[nodes].astype(np.float64)
        dh = dHn[nodes].astype(np.float64)
        order = np.argsort(-(dl + dh), kind="stable")
        bl = np.zeros(NSW)
        bh = np.zeros(NSW)
        bin_cnt = np.zeros(NSW, np.int64)
        for i in order:
            # joint L/H balance: minimize sum of squared per-stream loads
            cost = (bl + dl[i]) ** 2 + (bh + dh[i]) ** 2
            cost[bin_cnt >= P] = np.inf
            b = int(np.argmin(cost))
            slot = bin_cnt[b]
            bin_cnt[b] += 1
            bl[b] += dl[i]
            bh[b] += dh[i]
            n = nodes[i]
            pos_of_node[n] = c * NPU + b * P + slot
            node_at_pos[c][b * P + slot] = n

    # zero rows (a pad position in lo cores / hi cores)
    zlo = int(np.where(node_at_pos[0] < 0)[0][0])          # in core 0
    zhi_core = NCORES - 1
    zhi = int(np.where(node_at_pos[zhi_core] < 0)[0][0]) + zhi_core * NPU - B_SPLIT
    assert zlo < 32768 and 0 <= zhi < 32768

    # --- per-core edge tiling ---
    src_pg = pos_of_node[src]
    dst_pos = pos_of_node[dst]
    dst_core = node_core[dst]

    def pack_core(c, count_only_tt=None):
        """Build (or count) tiles for core c. Returns per-stream tile arrays."""
        em = dst_core == c
        es, ed, eL = src_pg[em], dst_pos[em] - c * NPU, e_isL[em]
        # append self edges for real nodes
        nodes = np.arange(S[c], S[c + 1])
        sp = pos_of_node[nodes]
        ss, sd = sp, sp - c * NPU
        sL = node_core[nodes] < HALF
        allsrc = np.concatenate([es, ss])
        alldst = np.concatenate([ed, sd])
        allL = np.concatenate([eL, sL])
        out = {}
        for stream, m in (("L", allL), ("H", ~allL)):
            ssrc = allsrc[m]
            sdst = alldst[m]
            o = np.argsort(sdst, kind="stable")
            ssrc, sdst = ssrc[o], sdst[o]
            if stream == "H":
                ssrc = ssrc - B_SPLIT
            zrow = zlo if stream == "L" else zhi
            # dense pack: edges of a subwindow fill tiles back-to-back (a
            # node's run may split across tiles — the Sel matmuls accumulate)
            sw_of = sdst // P
            rank_of = (sdst % P).astype(np.int16)
            tiles_idx, tiles_rank, tiles_sw = [], [], []
            bnd = np.searchsorted(sw_of, np.arange(NSW + 1))
            for sw in range(NSW):
                e0, e1 = int(bnd[sw]), int(bnd[sw + 1])
                k = e1 - e0
                nt = max(1, -(-k // P))
                if count_only_tt is not None:
                    tt = count_only_tt[stream]
                    assert nt <= tt, (c, stream, sw, nt, tt)
                    nt = tt
                buf_i = np.full(nt * P, zrow, np.int16)
                buf_r = np.zeros(nt * P, np.int16)
                buf_i[:k] = ssrc[e0:e1]
                buf_r[:k] = rank_of[e0:e1]
                tiles_idx.append(buf_i.reshape(nt, P))
                tiles_rank.append(buf_r.reshape(nt, P))
                tiles_sw.extend([sw] * nt)
            out[stream] = (np.concatenate(tiles_idx), np.concatenate(tiles_rank),
                           np.array(tiles_sw))
        return out

    # pass 1: find global TT per stream
    packed0 = [pack_core(c) for c in range(NCORES)]
    TT = {}
    for stream in ("L", "H"):
        mx = 0
        for c in range(NCORES):
            swl = packed0[c][stream][2]
            mx = max(mx, int(np.bincount(swl, minlength=NSW).max()))
        TT[stream] = mx
    # pass 2: uniform padding
    packed = [pack_core(c, count_only_tt=TT) for c in range(NCORES)]

    NT_L, NT_H = NSW * TT["L"], NSW * TT["H"]
    NCHUNK_L = int(np.ceil(NT_L / CH))
    NCHUNK_H = int(np.ceil(NT_H / CH))

    # --- tables (weight preprocessing) ---
    flat_atom = np.zeros((AC * AV + 16, D), np.float32)
    flat_atom[:AC * AV] = atom_emb.reshape(AC * AV, D)
    flat_atom_bf = flat_atom.astype(ml_dtypes.bfloat16)
    ZATOM = AC * AV

    # fused per-layer bond table [L, NCOMB, D]
    k = np.arange(NCOMB)
    d0, d1, d2 = k // (BV * BV), (k // BV) % BV, k % BV
    T512 = (edge_emb[:, 0, d0] + edge_emb[:, 1, d1] + edge_emb[:, 2, d2])
    T512_bf = T512.astype(ml_dtypes.bfloat16)        # [L, NCOMB, D]

    cidx = (efeat[:, 0] * BV + efeat[:, 1]) * BV + efeat[:, 2]

    cfg = dict(N=N, E=E, G=G, GPC=GPC, D=D, L=L, OUT=OUT, NPU=NPU, NSW=NSW,
               NWIN=nwin, TT_L=TT["L"], TT_H=TT["H"], NT_L=NT_L, NT_H=NT_H,
               NCHUNK_L=NCHUNK_L, NCHUNK_H=NCHUNK_H, B_SPLIT=B_SPLIT,
               NCOMB=NCOMB, AC=AC, ZATOM=ZATOM, NREAL=N)

    # --- per-core input maps ---
    in_maps = []
    for c in range(NCORES):
        m = {}
        for stream, nch in (("L", NCHUNK_L), ("H", NCHUNK_H)):
            ti, tr, _ = packed[c][stream]
            nt = ti.shape[0]
            pad_t = nch * CH - nt
            zrow = zlo if stream == "L" else zhi
            if pad_t:
                ti = np.concatenate([ti, np.full((pad_t, P), zrow, np.int16)])
                tr = np.concatenate([tr, np.zeros((pad_t, P), np.int16)])
            m[f"gidx{stream}"] = _wrap_idx(ti.reshape(-1))
            m[f"rank{stream}"] = tr.T.astype(ml_dtypes.bfloat16).copy()  # [P, nt]
        # countT [NCOMB, NPU] bf16
        em = dst_core == c
        lp = dst_pos[em] - c * NPU
        ct = np.zeros((NCOMB, NPU), np.float32)
        np.add.at(ct, (cidx[em], lp), 1.0)
        assert ct.max() <= 16, ct.max()
        m["countT"] = ct.astype(ml_dtypes.float8_e4m3)
        # deginv broadcast row [P, NPU]
        dg = np.zeros(NPU, np.float32)
        rp = node_at_pos[c] >= 0
        dg[rp] = deginv_all[node_at_pos[c][rp]]
        m["deginv"] = np.tile(dg[None, :], (P, 1))
        # mask [P, NSW]
        mk = rp.astype(np.float32).reshape(NSW, P).T.copy()
        m["maskc"] = mk
        # selpool [NPU, GPC]
        sp = np.zeros((NPU, GPC), np.float32)
        gl = np.where(rp)[0]
        gid = graph_ids[node_at_pos[c][gl]] - c * GPC
        cnts = np.maximum(gcnt[c * GPC:(c + 1) * GPC], 1.0)
        sp[gl, gid] = (1.0 / cnts[gid]).astype(np.float32)
        m["selpool"] = sp
        # layer-1 atom-count matrix [AC*AV, NPU]: for every edge with dst
        # here, count the src atom codes per column; +1 self entry per node.
        # Layer-1 messages then need NO gathers: neigh+self = cntA^T @ emb.
        es_ = src[em]
        own = node_at_pos[c][rp]
        srcs_all = np.concatenate([es_, own])
        dpos_all = np.concatenate([lp, np.where(rp)[0]])
        cA = np.zeros(AC * AV * NPU, np.float32)
        for col in range(AC):
            idxf = (col * AV + nfeat[srcs_all, col]) * NPU + dpos_all
            cA += np.bincount(idxf, minlength=AC * AV * NPU)
        cA = cA.reshape(AC * AV, NPU)
        assert cA.max() <= 16, cA.max()
        m["cntA"] = cA.astype(ml_dtypes.float8_e4m3)
        # atom-emb lhsT k-tiles [P(vocab), AC, KD, P(feat)]
        m["atomkt"] = np.ascontiguousarray(
            atom_emb.reshape(AC, AV, D // P, P).transpose(1, 0, 2, 3)
        ).astype(ml_dtypes.bfloat16)
        # W as [L, P, KD, D]: per-layer k-tiles ready for SBUF rhs layout
        KD_ = D // P
        m["wl"] = W.reshape(L, KD_, P, D).transpose(0, 2, 1, 3).astype(
            ml_dtypes.bfloat16)
        # T512 as [L, P, NKC, D] bf16 k-tiles
        NKC_ = NCOMB // P
        m["t512kt"] = np.ascontiguousarray(
            T512_bf.reshape(L, NKC_, P, D).transpose(0, 2, 1, 3))
        m["gam"] = gamma.reshape(L, 1, D).copy()
        m["bet"] = beta.reshape(L, 1, D).copy()
        m["wp"] = Wp.copy()                            # [D, OUT]
        m["bpr"] = bp.reshape(1, OUT).copy()
        m["iota_row"] = np.tile(np.arange(P, dtype=np.float32), (P, 1)).astype(
            ml_dtypes.bfloat16)
        in_maps.append(m)

    meta = dict(S=S, Nc=Nc)
    return cfg, in_maps, meta


# ----------------------------------------------------------------------------
# Device kernel builder (uniform SPMD program)
# ----------------------------------------------------------------------------

def build(cfg):
    D = cfg["D"]; L = cfg["L"]; NPU = cfg["NPU"]; NSW = cfg["NSW"]
    NWIN = cfg["NWIN"]; TT_L = cfg["TT_L"]; TT_H = cfg["TT_H"]
    NT_L = cfg["NT_L"]; NT_H = cfg["NT_H"]
    NCH_L = cfg["NCHUNK_L"]; NCH_H = cfg["NCHUNK_H"]
    NCOMB = cfg["NCOMB"]; AC = cfg["AC"]; GPC = cfg["GPC"]; OUT = cfg["OUT"]
    B_SPLIT = cfg["B_SPLIT"]; NREAL = cfg["NREAL"]
    ZROWS = 16
    KD = D // P            # feature k-tiles (2)
    SPW = WSZ // P         # subwindows per window (4)
    NKC = NCOMB // P       # count k-tiles (4)

    no_coll = os.environ.get("KGCN_NO_COLL") == "1"
    stage = int(os.environ.get("KGCN_STAGE", "5"))
    nostatic = os.environ.get("KGCN_NOSTATIC") == "1"
    nqueues = int(os.environ.get("KGCN_NQ", "4"))
    scratch = int(os.environ.get("KGCN_SCRATCH", "32768"))
    nc = bacc.Bacc("TRN2", target_bir_lowering=False, debug=False,
                   num_devices=NCORES, num_swdge_queues=nqueues,
                   dynamic_dma_scratch_size=scratch)

    # Round-robin gathers over the SWDGE queues: each queue has its own
    # descriptor ring, so desc-gen of gather N+1 proceeds while gather N's
    # descriptors drain to the DMA engines (one ring holds one 2048-desc
    # gather at scratch=32K).
    rrq = [0]

    def gather(out_ap, tbl_ap, idx_ap, nidx):
        nc.gpsimd.dma_gather(out_ap, tbl_ap, idx_ap, nidx, nidx, D,
                             single_packet=False, queue_num=rrq[0])
        rrq[0] = (rrq[0] + 1) % nqueues

    def allgather(ins, outs):
        if no_coll:
            nc.gpsimd.dma_start(out=outs[0][0:ins[0].shape[0], :], in_=ins[0])
        else:
            nc.gpsimd.collective_compute(
                "AllGather", mybir.AluOpType.bypass,
                replica_groups=[list(range(NCORES))], ins=ins, outs=outs)

    def allreduce(ins, outs):
        if no_coll:
            nc.gpsimd.dma_start(out=outs[0], in_=ins[0])
        else:
            nc.gpsimd.collective_compute(
                "AllReduce", mybir.AluOpType.add,
                replica_groups=[list(range(NCORES))], ins=ins, outs=outs)

    # external inputs
    t_gidxL = nc.dram_tensor("gidxL", [P, NCH_L * CH * P // 16], I16, kind="ExternalInput")
    t_gidxH = nc.dram_tensor("gidxH", [P, NCH_H * CH * P // 16], I16, kind="ExternalInput")
    t_rankL = nc.dram_tensor("rankL", [P, NCH_L * CH], BF16, kind="ExternalInput")
    t_rankH = nc.dram_tensor("rankH", [P, NCH_H * CH], BF16, kind="ExternalInput")
    t_countT = nc.dram_tensor("countT", [NCOMB, NPU], FP8, kind="ExternalInput")
    t_deginv = nc.dram_tensor("deginv", [P, NPU], F32, kind="ExternalInput")
    t_mask = nc.dram_tensor("maskc", [P, NSW], F32, kind="ExternalInput")
    t_selpool = nc.dram_tensor("selpool", [NPU, GPC], F32, kind="ExternalInput")
    t_cntA = nc.dram_tensor("cntA", [AC * 128, NPU], FP8, kind="ExternalInput")
    t_atomkt = nc.dram_tensor("atomkt", [P, AC, KD, P], BF16, kind="ExternalInput")
    t_t512 = nc.dram_tensor("t512kt", [L, P, NKC, D], BF16, kind="ExternalInput")
    t_wl = nc.dram_tensor("wl", [L, P, KD, D], BF16, kind="ExternalInput")
    t_gam = nc.dram_tensor("gam", [L, 1, D], F32, kind="ExternalInput")
    t_bet = nc.dram_tensor("bet", [L, 1, D], F32, kind="ExternalInput")
    t_wp = nc.dram_tensor("wp", [D, OUT], F32, kind="ExternalInput")
    t_bp = nc.dram_tensor("bpr", [1, OUT], F32, kind="ExternalInput")
    t_iota = nc.dram_tensor("iota_row", [P, P], BF16, kind="ExternalInput")
    # output
    t_out = nc.dram_tensor("out_g", [GPC, OUT], F32, kind="ExternalOutput")
    # internal DRAM
    fp8 = os.environ.get("KGCN_FP8", "1") == "1"
    HDT = FP8 if fp8 else BF16
    shared = os.environ.get("KGCN_SHARED", "1") == "1"
    t_hfull = nc.dram_tensor("h_full", [NCORES * NPU, D], HDT,
                             addr_space="Shared" if shared else "Local")
    t_hnew = nc.dram_tensor("h_newc", [NPU, D], HDT)
    t_arin = [nc.dram_tensor(f"arin{l}", [2, D], F32) for l in range(L)]
    t_arout = [nc.dram_tensor(f"arout{l}", [2, D], F32) for l in range(L)]

    with tile.TileContext(nc) as tc:
        with (
            tc.tile_pool(name="static", bufs=1) as stp,
            tc.tile_pool(name="gath", bufs=3) as gpool,
            tc.tile_pool(name="selp", bufs=3) as selpool_p,
            tc.tile_pool(name="xt", bufs=4) as xtp,
            tc.tile_pool(name="work", bufs=3) as wk,
            tc.tile_pool(name="small", bufs=1) as smp,
            tc.tile_pool(name="winps", bufs=2, space="PSUM") as wps,
            tc.tile_pool(name="hlps", bufs=2, space="PSUM") as hps,
            tc.tile_pool(name="smps", bufs=1, space="PSUM") as sps,
            tc.tile_pool(name="abps", bufs=1, space="PSUM") as aps,
        ):

            # ---- static SBUF preloads ----
            gidxL = stp.tile([P, NCH_L * CH * P // 16], I16)
            gidxH = stp.tile([P, NCH_H * CH * P // 16], I16)
            rankL = stp.tile([P, NCH_L * CH], BF16)
            rankH = stp.tile([P, NCH_H * CH], BF16)
            atomS = stp.tile([P, AC, KD, P], BF16)
            maskS = stp.tile([P, NSW], F32)
            selpS = stp.tile([P, NSW, GPC], F32)
            hlinS = stp.tile([P, NSW, D], BF16)        # pre-BN h, SBUF-resident
            gamS = stp.tile([1, L, D], F32)
            betS = stp.tile([1, L, D], F32)
            wpS = stp.tile([P, KD, OUT], F32)
            bpS = stp.tile([1, OUT], F32)
            onesS = stp.tile([1, P], F32)
            iotaS = stp.tile([P, P], BF16)
            if not nostatic:
                nc.sync.dma_start(iotaS[:], t_iota[:])
            epsS = stp.tile([1, 1], F32)
            nc.vector.memset(epsS[:], EPS)
            if not nostatic:
                nc.sync.dma_start(gidxL[:], t_gidxL[:])
            if not nostatic:
                nc.sync.dma_start(gidxH[:], t_gidxH[:])
            if not nostatic:
                nc.sync.dma_start(rankL[:], t_rankL[:])
            if not nostatic:
                nc.sync.dma_start(rankH[:], t_rankH[:])
            nc.sync.dma_start(atomS[:], t_atomkt[:])
            if not nostatic:
                nc.sync.dma_start(maskS[:], t_mask[:])
            if not nostatic:
                nc.sync.dma_start(selpS[:], t_selpool.ap().rearrange("(s p) g -> p s g", p=P))
            if not nostatic:
                nc.sync.dma_start(gamS[:], t_gam.ap().rearrange("l o d -> o l d"))
            if not nostatic:
                nc.sync.dma_start(betS[:], t_bet.ap().rearrange("l o d -> o l d"))
            if not nostatic:
                nc.sync.dma_start(wpS[:], t_wp.ap().rearrange("(k p) o -> p k o", p=P))
            if not nostatic:
                nc.sync.dma_start(bpS[:], t_bp[:])
            nc.vector.memset(onesS[:], 1.0)

            # ================= layers =================
            for l in range(L if stage >= 2 else 0):
                wl_t = wk.tile([P, KD, D], BF16, tag="wl", bufs=2)
                nc.sync.dma_start(wl_t[:], t_wl[l])
                t5l = wk.tile([P, NKC, D], BF16, tag="t5", bufs=2)
                nc.sync.dma_start(t5l[:], t_t512[l])
                stats0 = sps.tile([1, D], F32, tag="stats0")
                stats1 = sps.tile([1, D], F32, tag="stats1")
                if l == L - 1:
                    poolps = [sps.tile([P, GPC], F32, tag=f"pool{h}",
                                       name=f"pool{h}") for h in range(KD)]
                # --- windows: messages + encoder + x^T + update + stats ---
                for w in range(NWIN):
                    winp = [wps.tile([P, WSZ], F32, tag="win", name=f"win{h}")
                            for h in range(KD)]
                    ctks = []
                    for kk in range(NKC):
                        ctk = wk.tile([P, WSZ], FP8, tag="ct", name=f"ct{kk}",
                                      bufs=2 * NKC)
                        nc.sync.dma_start(
                            ctk[:], t_countT[kk * P:(kk + 1) * P,
                                             w * WSZ:(w + 1) * WSZ])
                        ctks.append(ctk)
                    # window-wide bond-encoder matmuls open the PSUM
                    # accumulation (independent of the gathers, so they
                    # schedule early and hide gather latency)
                    for kk in range(NKC):
                        for h in range(KD):
                            nc.tensor.matmul(
                                out=winp[h][:],
                                lhsT=t5l[:, kk, h * P:(h + 1) * P],
                                rhs=ctks[kk][:],
                                start=(kk == 0), stop=False)
                    if l == 0:
                        # layer-1 fusion: atom-code count matmuls supply the
                        # h0[src] message sums + self term — no gathers at all
                        cas = []
                        for ci in range(AC):
                            cat = wk.tile([P, WSZ], FP8, tag="ca",
                                          name=f"ca{ci}", bufs=2 * AC)
                            nc.sync.dma_start(
                                cat[:], t_cntA[ci * P:(ci + 1) * P,
                                               w * WSZ:(w + 1) * WSZ])
                            cas.append(cat)
                        for ci in range(AC):
                            for h in range(KD):
                                nc.tensor.matmul(
                                    out=winp[h][:],
                                    lhsT=atomS[:, ci, h, :],
                                    rhs=cas[ci][:],
                                    start=False, stop=(ci == AC - 1))
                    for sw in range(SPW if l > 0 else 0):
                        gsw = w * SPW + sw
                        for stream, tt, gidx, rank in (
                                ("L", TT_L, gidxL, rankL),
                                ("H", TT_H, gidxH, rankH)):
                            for t in range(tt):
                                tg = gsw * tt + t          # global tile no.
                                chk, off = tg // CH, tg % CH
                                if off == 0:
                                    # issue bulk gather + batched is_equal
                                    gt = gpool.tile([P, CH, D], HDT,
                                                    tag=f"g{stream}",
                                                    name=f"g{stream}t",
                                                    bufs=8 if stream == "L" else 6)
                                    nidx = CH * P
                                    tbl = (t_hfull[0:B_SPLIT, :] if stream == "L"
                                           else t_hfull[B_SPLIT:NCORES * NPU, :])
                                    gather(gt[:], tbl,
                                           gidx[:, chk * (nidx // 16):(chk + 1) * (nidx // 16)],
                                           nidx)
                                    sel = selpool_p.tile([P, CH, P],
                                                         FP8 if fp8 else BF16,
                                                         tag=f"s{stream}",
                                                         name=f"s{stream}t")
                                    rk = rank[:, chk * CH:(chk + 1) * CH]
                                    in0 = bass.AP(rk.tensor, rk.offset,
                                                  [rk.ap[0], list(rk.ap[1]), [0, P]])
                                    io = iotaS[:]
                                    in1 = bass.AP(io.tensor, io.offset,
                                                  [io.ap[0], [0, CH], [1, P]])
                                    nc.vector.tensor_tensor(
                                        out=sel[:], in0=in0, in1=in1,
                                        op=mybir.AluOpType.is_equal)
                                    if stream == "L":
                                        curL_g, curL_s = gt, sel
                                    else:
                                        curH_g, curH_s = gt, sel
                                gt, sel = ((curL_g, curL_s) if stream == "L"
                                           else (curH_g, curH_s))
                                last = (stream == "H" and t == tt - 1)
                                for h in range(KD):
                                    nc.tensor.matmul(
                                        out=winp[h][:, sw * P:(sw + 1) * P],
                                        lhsT=gt[:, off, h * P:(h + 1) * P],
                                        rhs=sel[:, off, :],
                                        start=False, stop=last)
                    # x^T = deginv * window
                    dgt = wk.tile([P, WSZ], F32, tag="dg")
                    nc.sync.dma_start(dgt[:], t_deginv[:, w * WSZ:(w + 1) * WSZ])
                    xt = [xtp.tile([P, WSZ], BF16, tag="xt", name=f"xt{h}")
                          for h in range(KD)]
                    for h in range(KD):
                        nc.vector.tensor_tensor(out=xt[h][:], in0=winp[h][:],
                                                in1=dgt[:],
                                                op=mybir.AluOpType.mult)
                    # update matmul + stats per subtile
                    for sw in range(SPW if stage >= 3 else 0):
                        st = w * SPW + sw
                        hlp = hps.tile([P, D], F32, tag="hl")
                        for h in range(KD):
                            nc.tensor.matmul(
                                out=hlp[:],
                                lhsT=xt[h][:, sw * P:(sw + 1) * P],
                                rhs=wl_t[:, h, :],
                                start=(h == 0), stop=(h == KD - 1))
                        hls = wk.tile([P, D], F32, tag="hls")
                        nc.scalar.activation(hls[:], hlp[:],
                                             mybir.ActivationFunctionType.Copy)
                        nc.scalar.activation(hlinS[:, st, :], hlp[:],
                                             mybir.ActivationFunctionType.Copy)
                        sq = wk.tile([P, D], F32, tag="sq")
                        nc.vector.tensor_tensor(out=sq[:], in0=hls[:], in1=hls[:],
                                                op=mybir.AluOpType.mult)
                        nc.tensor.matmul(out=stats0[:],
                                         lhsT=maskS[:, st:st + 1], rhs=hls[:],
                                         start=(st == 0), stop=(st == NSW - 1))
                        nc.tensor.matmul(out=stats1[:],
                                         lhsT=maskS[:, st:st + 1], rhs=sq[:],
                                         start=(st == 0), stop=(st == NSW - 1))
                # --- BN stats allreduce + scale/shift ---
                if stage < 3:
                    continue
                stsb0 = smp.tile([1, D], F32, tag="stsb0")
                stsb1 = smp.tile([1, D], F32, tag="stsb1")
                nc.scalar.activation(stsb0[:], stats0[:],
                                     mybir.ActivationFunctionType.Copy)
                nc.scalar.activation(stsb1[:], stats1[:],
                                     mybir.ActivationFunctionType.Copy)
                nc.sync.dma_start(t_arin[l][0:1, :], stsb0[:])
                nc.sync.dma_start(t_arin[l][1:2, :], stsb1[:])
                allreduce([t_arin[l][:]], [t_arout[l][:]])
                stg0 = smp.tile([1, D], F32, tag="stg0")
                stg1 = smp.tile([1, D], F32, tag="stg1")
                nc.sync.dma_start(stg0[:], t_arout[l][0:1, :])
                nc.sync.dma_start(stg1[:], t_arout[l][1:2, :])
                mean = smp.tile([1, D], F32, tag="mean")
                nc.vector.tensor_scalar_mul(mean[:], stg0[:], 1.0 / NREAL)
                msq = smp.tile([1, D], F32, tag="msq")
                nc.vector.tensor_scalar_mul(msq[:], stg1[:], 1.0 / NREAL)
                var = smp.tile([1, D], F32, tag="var")
                nc.vector.tensor_tensor(out=var[:], in0=mean[:], in1=mean[:],
                                        op=mybir.AluOpType.mult)
                nc.vector.tensor_tensor(out=var[:], in0=msq[:], in1=var[:],
                                        op=mybir.AluOpType.subtract)
                sd = smp.tile([1, D], F32, tag="sd")
                nc.scalar.activation(sd[:], var[:],
                                     mybir.ActivationFunctionType.Sqrt,
                                     bias=epsS[:])
                rsq = smp.tile([1, D], F32, tag="rsq")
                nc.vector.reciprocal(rsq[:], sd[:])
                abin = smp.tile([1, 2 * D], F32, tag="abin")
                scl = abin[:, 0:D]
                nc.vector.tensor_tensor(out=scl, in0=rsq[:],
                                        in1=gamS[:, l, :],
                                        op=mybir.AluOpType.mult)
                sft = abin[:, D:2 * D]
                nc.vector.tensor_tensor(out=sft, in0=mean[:], in1=scl,
                                        op=mybir.AluOpType.mult)
                nc.vector.tensor_tensor(out=sft, in0=betS[:, l, :],
                                        in1=sft,
                                        op=mybir.AluOpType.subtract)
                # broadcast coef rows to all partitions on the (idle) gpsimd
                # engine; keeps ab in SBUF and frees a PSUM bank for hps=2
                ab = smp.tile([P, 2 * D], F32, tag="ab")
                nc.gpsimd.partition_broadcast(ab[:], abin[:])
                # --- apply + (layer L-1) pooling ---
                for st in range(NSW if stage >= 4 else 0):
                    hnf = wk.tile([P, D], F32, tag="hnf")
                    nc.vector.tensor_tensor(out=hnf[:], in0=hlinS[:, st, :],
                                            in1=ab[:, 0:D],
                                            op=mybir.AluOpType.mult)
                    nc.vector.tensor_tensor(out=hnf[:], in0=hnf[:],
                                            in1=ab[:, D:2 * D],
                                            op=mybir.AluOpType.add)
                    # relu + pad-row mask in one scalar-engine op:
                    # mask is per-node (= per-partition), so it rides the
                    # activation scale; relu(x)*m == relu(x*m) for m in {0,1}
                    if l < L - 1:
                        hnb = wk.tile([P, D], HDT, tag="hnb")
                        nc.scalar.activation(hnb[:], hnf[:],
                                             mybir.ActivationFunctionType.Relu,
                                             scale=maskS[:, st:st + 1])
                        nc.sync.dma_start(t_hnew[st * P:(st + 1) * P, :], hnb[:])
                    else:
                        # pooling ignores pad rows (selpool rows are zero),
                        # so plain relu suffices
                        nc.vector.tensor_scalar_max(hnf[:], hnf[:], 0.0)
                        for h in range(KD):
                            nc.tensor.matmul(
                                out=poolps[h][:],
                                lhsT=hnf[:, h * P:(h + 1) * P],
                                rhs=selpS[:, st, :],
                                start=(st == 0), stop=(st == NSW - 1))
                if l < L - 1:
                    allgather([t_hnew[:]], [t_hfull[:]])

            # ================= readout =================
            if stage < 5:
                dummy = smp.tile([GPC, OUT], F32, tag="dummy")
                nc.vector.memset(dummy[:], 0.0)
                nc.sync.dma_start(t_out[:], dummy[:])
            else:
                gts = smp.tile([P, KD * GPC], F32, tag="gts")
                for h in range(KD):
                    nc.scalar.activation(gts[:, h * GPC:(h + 1) * GPC],
                                         poolps[h][:],
                                         mybir.ActivationFunctionType.Copy)
                ones16 = smp.tile([1, GPC], F32, tag="o16")
                nc.vector.memset(ones16[:], 1.0)
                outp = sps.tile([GPC, OUT], F32, tag="stats0")
                for h in range(KD):
                    nc.tensor.matmul(out=outp[:],
                                     lhsT=gts[:, h * GPC:(h + 1) * GPC],
                                     rhs=wpS[:, h, :], start=(h == 0), stop=False)
                nc.tensor.matmul(out=outp[:], lhsT=ones16[:], rhs=bpS[:],
                                 start=False, stop=True)
                outs = smp.tile([GPC, OUT], F32, tag="outs")
                nc.scalar.activation(outs[:], outp[:],
                                     mybir.ActivationFunctionType.Copy)
                nc.sync.dma_start(t_out[:], outs[:])

    nc.compile()
    return nc


LAST = {}


def kernel(**inputs):
    cfg, in_maps, _ = preprocess(inputs)
    nc = build(cfg)
    trace = os.environ.get("KGCN_TRACE") == "1"
    res = run_bass_kernel_spmd(nc, in_maps, list(range(NCORES)), trace=trace,
                               tmpdir=os.environ.get("KGCN_TMPDIR"))
    LAST["exec_time_ns"] = res.exec_time_ns
    LAST["profile_json"] = res.profile_json
    out = np.concatenate([res.results[c]["out_g"] for c in range(NCORES)], 0)
    return out.astype(np.float32)


if __name__ == "__main__":
    pass

